# revision 24
# baseline (speedup 1.0000x reference)
"""Bit2Num dequantization kernel for Trainium2 (8 NeuronCores, SPMD).

Reference op: x [1024, 65536] of {0.0, 1.0} f32, B=4.
  bits = x.reshape(1024, 16384, 4)
  out[b, n] = (8*bits[b,n,0] + 4*bits[b,n,1] + 2*bits[b,n,2] + bits[b,n,3] + 0.5) / 16

Sharding: pure data-parallel over batch — 128 rows per core (= 128 SBUF
partitions). Per core: 32 MiB f32 in + 4 MiB bf16 out. The 16 SDMA
engines/core run at ~27.2 GB/s each (engine time ~ max(m2s, s2m) bytes),
so the f32 in-stream sets the floor: ~77 us of engine time + ~8.6 us
fixed startup + ~3 us close-out.

Per-core kernel, pipelined over 1 MiB column segments of [128, 2048]:
  - Loads on the SP HWDGE ring (nc.sync, plain f32). HWDGE completions
    are plain HW sems the consumer (DVE) waits on directly; SWDGE
    (gpsimd-cast) loads instead round-trip the GpSimd sequencer
    (gen -> wait -> event broadcast) which head-of-line serializes the
    pipeline at ~3 us/segment.
  - 3 scalar_tensor_tensor ops on DVE per segment (u=2a+b, v=2c+d,
    w=4u+v over the 4 strided bit slices), bf16 intermediates (exact:
    all values <= 15).
  - ACT does the affine (w/16 + 1/32) with bf16 output and issues the
    store on its own HWDGE ring (qActDynamicHW), so stores never sit in
    the load FIFO.
  - Output is STORED bf16: every output value is (2k+1)/32, k=0..15 —
    exact in bf16 — and the host upcasts to f32 during the gather.
    Halves store-side HBM traffic vs f32.
  - Tapered trailing segments shrink the exposed compute/store tail
    after the last load lands.
Measured: bit-exact; ~107 us span on a quiet core (run-to-run spread up
to ~+20% from HBM-stack sharing with the partner core).
"""

import numpy as np

import concourse.bacc as bacc
import concourse.bass as bass
import concourse.mybir as mybir
from concourse.bass_utils import run_bass_kernel_spmd
from concourse.tile import TileContext

N_CORES = 8
BATCH = 1024
COLS = 65536
B_BITS = 4
ROWS = BATCH // N_CORES          # 128 rows per core == SBUF partition count
OUT_COLS = COLS // B_BITS        # 16384

F32 = mybir.dt.float32
BF16 = mybir.dt.bfloat16
MULT = mybir.AluOpType.mult
ADD = mybir.AluOpType.add


def _build_nc() -> bass.Bass:
    # Bacc (not plain Bass): its compile() pipeline runs
    # generate_event_semaphores, which splits multi-wait sync conditions —
    # TRN2 DMA instructions accept at most one wait.
    nc = bacc.Bacc(None, target_bir_lowering=False)
    x = nc.dram_tensor("x", [ROWS, COLS], F32, kind="ExternalInput")
    # Output is stored bf16: every output value is (2k+1)/32, k=0..15 —
    # exactly representable in bf16 (<=5 significand bits). Halves the
    # store-side HBM traffic; host upcasts to f32 during the gather.
    out = nc.dram_tensor("out", [ROWS, OUT_COLS], BF16, kind="ExternalOutput")

    # Segment list (in-DMA column widths + per-segment compute chunks).
    # 2048 f32 cols = 1 MiB per load; the tail tapers to 1024-col segments
    # to shrink the compute/store chain exposed after the last load.
    # Do NOT taper below 1024 cols: a 128-g bf16 store is 256 B/partition,
    # under the 512 B SDMA minimum — adjacent stores then read-modify-write
    # the same granule concurrently and corrupt the output (measured).
    segments = [(2048, [512])] * 30 + [(1024, [256])] * 4
    assert sum(s[0] for s in segments) == COLS

    with TileContext(nc) as tc:
        with (
            # bufs=8 on the input pool keeps the load ring ~8 segments
            # ahead of compute; work/out pools at 4 keep buffer-recycle
            # waits (store receipts) off the critical path.
            tc.tile_pool(name="xin", bufs=8) as xpool,
            tc.tile_pool(name="work", bufs=6) as wpool,
            tc.tile_pool(name="oout", bufs=6) as opool,
        ):
            col = 0
            g_off = 0
            for seg_c, chunk_gs in segments:
                xt = xpool.tile([ROWS, seg_c], F32, tag="xt")
                # HWDGE in-DMAs on the Sync ring (f32, no cast): SWDGE
                # completion must round-trip through the GpSimd sequencer
                # (gen -> wait -> event broadcast), which serializes the
                # whole pipeline at ~3 us/segment. HWDGE completions are
                # plain HW sems the consumer waits on directly.
                nc.sync.dma_start(
                    out=xt[:, :], in_=x[:, col:col + seg_c]
                )
                col += seg_c
                c_off = 0
                for chunk_g in chunk_gs:
                    chunk_c = chunk_g * B_BITS
                    xp = xt[:, c_off:c_off + chunk_c].rearrange(
                        "p (g k) -> p g k", k=2
                    )
                    c_off += chunk_c

                    # intermediates stay bf16 (all values <= 15, exact);
                    # ACT casts on the final affine.
                    t = wpool.tile([ROWS, 2 * chunk_g], BF16, tag="t")
                    w = wpool.tile([ROWS, chunk_g], BF16, tag="w")
                    ot = opool.tile([ROWS, chunk_g], BF16, tag="ot")

                    # Pairwise tree in 2 DVE ops instead of 3 (same element
                    # count, one less per-op fixed cost; DVE runs 1x on the
                    # strided slices either way). Keeps DVE busy ~62 us vs
                    # ~86 us — margin so DVE tracks the load frontier and
                    # the end-of-stream compute tail stays short.
                    #   t = 2*x_even + x_odd   -> interleaved [u, v]
                    #   w = 4*t_even + t_odd   = 8a+4b+2c+d
                    nc.vector.scalar_tensor_tensor(
                        out=t[:, :], in0=xp[:, :, 0], scalar=2.0,
                        in1=xp[:, :, 1], op0=MULT, op1=ADD,
                    )
                    tv = t[:, :].rearrange("p (g k) -> p g k", k=2)
                    nc.vector.scalar_tensor_tensor(
                        out=w[:, :], in0=tv[:, :, 0], scalar=4.0,
                        in1=tv[:, :, 1], op0=MULT, op1=ADD,
                    )
                    # ot = (w + 0.5) / 16 = w/16 + 1/32
                    nc.scalar.activation(
                        out=ot[:, :], in_=w[:, :],
                        func=mybir.ActivationFunctionType.Copy,
                        bias=1.0 / 32.0, scale=1.0 / 16.0,
                    )
                    # out-DMAs on the ACT HWDGE ring (qActDynamicHW) so a
                    # store waiting on compute never blocks the in-stream.
                    nc.scalar.dma_start(
                        out=out[:, g_off:g_off + chunk_g], in_=ot[:, :]
                    )
                    g_off += chunk_g
    # Bacc.finalize runs the compile pipeline (register allocation +
    # generate_event_semaphores); the pjrt exec path serializes nc.m as-is.
    nc.finalize()
    return nc


_NC = None


def _get_nc() -> bass.Bass:
    global _NC
    if _NC is None:
        _NC = _build_nc()
    return _NC


def kernel(x: np.ndarray, B=4) -> np.ndarray:
    assert int(B) == B_BITS, f"kernel hardcodes B={B_BITS}, got {B}"
    x = np.ascontiguousarray(x, dtype=np.float32)
    assert x.shape == (BATCH, COLS), x.shape
    nc = _get_nc()
    in_maps = [{"x": x[i * ROWS:(i + 1) * ROWS]} for i in range(N_CORES)]
    res = run_bass_kernel_spmd(nc, in_maps, list(range(N_CORES)))
    return np.concatenate(
        [res.results[i]["out"] for i in range(N_CORES)], axis=0
    ).astype(np.float32)

